# revision 2
# baseline (speedup 1.0000x reference)
"""Causal depthwise temporal conv (K=4) on 8 TRN2 NeuronCores.

Reference semantics (for x: [B, T, D], w: [K, D], b: [D]):
    out[bt, t, d] = sum_{j=0}^{K-1} x_pad[bt, t + j, d] * w[j, d] + b[d]
where x_pad is x left-padded with K-1 zeros along time.

Strategy (v2, fp16):
  - Tensor-parallel over channels: core m owns channels [m*512, (m+1)*512).
  - All HBM traffic in fp16 (harness gate is rel_err < 2e-2; fp16 keeps
    max-rel error ~1e-3): halves DMA bytes vs f32 -> ~94us roofline/core.
  - Host pre-transposes each core's shard to channel-major [D_sh, B, T+K-1]
    (left zero-padded, fp16). Channels sit on SBUF partitions so per-channel
    weights are per-partition scalars; temporal shifts are free-dim slices.
  - Engine split per (channel-block, batch) chain: the odd-shift taps (j=1,3)
    are 2B-misaligned in fp16 so they go to ACT (alignment-insensitive);
    the even-shift taps (j=0,2) + final combine go to DVE, whose fp16
    2x perf mode requires 4B-aligned step-1 APs.
"""

import numpy as np

import concourse.bacc as bacc
import concourse.mybir as mybir
from concourse.tile import TileContext
from concourse import bass_utils

B = 4            # batch
T = 4096         # sequence length
D = 4096         # channels (width)
K = 4            # temporal taps
N_CORES = 8
D_SH = D // N_CORES          # 512 channels per core
P = 128                      # SBUF partitions
N_BLK = D_SH // P            # 4 channel blocks per core
TP = T + K - 1               # padded time length
W_STRIDE = K + 1             # per-blk slot in the wb tile


def _build(b=B, t=T, n_blk=N_BLK):
    nc = bacc.Bacc("TRN2")
    tp = t + K - 1
    f16 = mybir.dt.float16
    f32 = mybir.dt.float32
    x = nc.dram_tensor("x", [n_blk, P, b, tp], f16, kind="ExternalInput")
    wb = nc.dram_tensor("wb", [P, n_blk * W_STRIDE], f32, kind="ExternalInput")
    out = nc.dram_tensor("out", [n_blk, P, b, t], f16, kind="ExternalOutput")
    mult, add = mybir.AluOpType.mult, mybir.AluOpType.add
    ident = mybir.ActivationFunctionType.Identity
    copy_fn = mybir.ActivationFunctionType.Copy

    with TileContext(nc) as tc:
        with tc.tile_pool(name="px", bufs=4) as px, \
             tc.tile_pool(name="ps", bufs=3) as ps, \
             tc.tile_pool(name="pa", bufs=3) as pa, \
             tc.tile_pool(name="pw", bufs=1) as pw:
            wt = pw.tile([P, n_blk * W_STRIDE], f32, tag="wb")
            nc.sync.dma_start(wt[:], wb[:, :])
            for blk in range(n_blk):
                def w(j, blk=blk):
                    return wt[:, blk * W_STRIDE + j:blk * W_STRIDE + j + 1]
                for bb in range(b):
                    X = px.tile([P, tp], f16, tag="x")
                    nc.sync.dma_start(X[:], x[blk, :, bb, :])
                    # ACT: the two misaligned (odd-shift) taps as partials.
                    s1 = ps.tile([P, t], f16, tag="s1")
                    nc.scalar.activation(s1[:], X[:, 1:1 + t], ident,
                                         bias=w(K), scale=w(1))
                    s3 = ps.tile([P, t], f16, tag="s3")
                    nc.scalar.activation(s3[:], X[:, 3:3 + t], copy_fn,
                                         bias=0.0, scale=w(3))
                    # DVE: aligned taps fused into the partials, then combine.
                    a = pa.tile([P, t], f16, tag="a")
                    nc.vector.scalar_tensor_tensor(
                        a[:], X[:, 0:t], w(0), s1[:], mult, add)
                    a2 = pa.tile([P, t], f16, tag="a2")
                    nc.vector.scalar_tensor_tensor(
                        a2[:], X[:, 2:2 + t], w(2), a[:], mult, add)
                    o = pa.tile([P, t], f16, tag="o")
                    # One chain on gpsimd to measure its fp16 TT rate.
                    eng = nc.gpsimd if (blk == n_blk - 1 and bb == b - 1) \
                        else nc.vector
                    eng.tensor_tensor(o[:], a2[:], s3[:], add)
                    nc.sync.dma_start(out[blk, :, bb, :], o[:])
    nc.compile()
    return nc


def _prepare(x, w, b):
    x = np.asarray(x, dtype=np.float32)
    w = np.asarray(w, dtype=np.float32)
    b = np.asarray(b, dtype=np.float32)
    # channel-major, left zero-padded time, fp16: [D, B, TP]
    xp = np.zeros((D, B, TP), dtype=np.float16)
    xp[:, :, K - 1:] = x.transpose(2, 0, 1)
    wbt = np.concatenate([w.T, b[:, None]], axis=1).astype(np.float32)  # [D, K+1]
    in_maps = []
    for m in range(N_CORES):
        sl = slice(m * D_SH, (m + 1) * D_SH)
        wbm = wbt[sl].reshape(N_BLK, P, W_STRIDE).transpose(1, 0, 2)
        in_maps.append({
            "x": np.ascontiguousarray(xp[sl]).reshape(N_BLK, P, B, TP),
            "wb": np.ascontiguousarray(wbm).reshape(P, N_BLK * W_STRIDE),
        })
    return in_maps


def _collect(results):
    out = np.empty((B, T, D), dtype=np.float32)
    for m in range(N_CORES):
        o = np.asarray(results[m]["out"]).astype(np.float32)
        o = o.reshape(D_SH, B, T)
        out[:, :, m * D_SH:(m + 1) * D_SH] = o.transpose(1, 2, 0)
    return out


def _run(in_maps, trace=False, **kwargs):
    nc = _build()
    return bass_utils.run_bass_kernel_spmd(
        nc, in_maps, core_ids=list(range(N_CORES)), trace=trace, **kwargs)


def kernel(x, w, b):
    in_maps = _prepare(x, w, b)
    try:
        res = _run(in_maps)
    except Exception:
        # Transient NRT device errors have been observed on a cold first
        # execute; one retry (fresh compile dir) clears them.
        res = _run(in_maps)
    return _collect(res.results)


# revision 3
# speedup vs baseline: 1.6264x; 1.6264x over previous
"""Causal depthwise temporal conv (K=4) on 8 TRN2 NeuronCores.

Reference semantics (for x: [B, T, D], w: [K, D], b: [D]):
    out[bt, t, d] = sum_{j=0}^{K-1} x_pad[bt, t + j, d] * w[j, d] + b[d]
where x_pad is x left-padded with K-1 zeros along time.

Strategy (v3, fp16 + PE accumulation):
  - Tensor-parallel over channels: core m owns channels [m*512, (m+1)*512).
  - All HBM traffic in fp16 (harness gate is rel_err < 2e-2; fp16 keeps
    max-rel error ~1e-3): halves DMA bytes vs f32 -> ~94us/core roofline.
  - Host pre-transposes each core's shard to channel-major [D_sh, B, T+K-1]
    (left zero-padded, fp16). Channels sit on SBUF partitions.
  - The 3 combining adds are the expensive part on DVE (tensor_tensor is
    2x-mode at best; scalar_tensor_tensor is 1x-only, measured). So taps
    {0,1,3} run on the tensor engine as diagonal-matrix matmuls that
    accumulate FOR FREE in PSUM (diag(w_j).T @ x_shift_j). ACT evacuates
    PSUM -> SBUF fp16 fused with the bias add (ACT is 1x but alignment- and
    dtype-insensitive). DVE only does tap 2 (tensor_scalar, 4x mode) and
    one tensor_tensor combine (2x mode).
  - PSUM ping-pong: each (blk, batch) chain is split into two 2048-col
    halves; each half uses 4 PSUM banks, bufs=2 covers all 8 banks.
"""

import numpy as np

import concourse.bacc as bacc
import concourse.mybir as mybir
from concourse.tile import TileContext
from concourse import bass_utils

B = 4            # batch
T = 4096         # sequence length
D = 4096         # channels (width)
K = 4            # temporal taps
N_CORES = 8
D_SH = D // N_CORES          # 512 channels per core
P = 128                      # SBUF partitions
N_BLK = D_SH // P            # 4 channel blocks per core
TP = T + K - 1               # padded time length
W_STRIDE = K + 1             # per-blk slot in the f32 wb tile

PE_TAPS = (0, 1, 3)          # taps done as diag-matmuls into PSUM
DVE_TAPS = (2,)              # taps done on DVE (must be even shifts: 4B align)
HALF = 2048                  # psum half-chain width (4 banks)
NB = 512                     # matmul moving-block width (1 psum bank)


def _build(b=B, t=T, n_blk=N_BLK):
    nc = bacc.Bacc("TRN2")
    tp = t + K - 1
    f16 = mybir.dt.float16
    f32 = mybir.dt.float32
    npe = len(PE_TAPS)
    x = nc.dram_tensor("x", [n_blk, P, b, tp], f16, kind="ExternalInput")
    wd = nc.dram_tensor("wd", [P, n_blk * npe * P], f16, kind="ExternalInput")
    wb = nc.dram_tensor("wb", [P, n_blk * W_STRIDE], f32, kind="ExternalInput")
    out = nc.dram_tensor("out", [n_blk, P, b, t], f16, kind="ExternalOutput")
    mult, add = mybir.AluOpType.mult, mybir.AluOpType.add
    ident = mybir.ActivationFunctionType.Identity
    nhalf = t // HALF

    with TileContext(nc) as tc:
        with tc.tile_pool(name="px", bufs=4) as px, \
             tc.tile_pool(name="ps", bufs=4) as ps, \
             tc.tile_pool(name="po", bufs=4) as po, \
             tc.tile_pool(name="pw", bufs=1) as pw, \
             tc.tile_pool(name="pp", bufs=2, space="PSUM") as pp:
            wdt = pw.tile([P, n_blk * npe * P], f16, tag="wd")
            nc.sync.dma_start(wdt[:], wd[:, :])
            wt = pw.tile([P, n_blk * W_STRIDE], f32, tag="wb")
            nc.sync.dma_start(wt[:], wb[:, :])

            for blk in range(n_blk):
                def w(j, blk=blk):
                    return wt[:, blk * W_STRIDE + j:blk * W_STRIDE + j + 1]

                def wdiag(ti, blk=blk):
                    o = (blk * npe + ti) * P
                    return wdt[:, o:o + P]

                for bb in range(b):
                    X = px.tile([P, tp], f16, tag="x")
                    nc.sync.dma_start(X[:], x[blk, :, bb, :])
                    for h in range(nhalf):
                        base = h * HALF
                        pt = pp.tile([P, HALF], f32, tag="ps")
                        # PE: psum[c] += sum_{j in PE_TAPS} diag(w_j) @ x_j
                        for c in range(HALF // NB):
                            lo = base + c * NB
                            for ti in range(npe):
                                nc.tensor.matmul(
                                    pt[:, c * NB:(c + 1) * NB],
                                    wdiag(ti),
                                    X[:, lo + PE_TAPS[ti]:lo + PE_TAPS[ti] + NB],
                                    start=(ti == 0),
                                    stop=(ti == npe - 1),
                                )
                        # ACT: evacuate PSUM -> fp16, fused bias add.
                        s = ps.tile([P, HALF], f16, tag="s")
                        nc.scalar.activation(s[:], pt[:, :], ident,
                                             bias=w(K), scale=1.0)
                        # DVE: remaining even taps + one combine (2x / 4x modes).
                        y = ps.tile([P, HALF], f16, tag="y")
                        j0 = DVE_TAPS[0]
                        nc.vector.tensor_scalar_mul(
                            y[:], X[:, base + j0:base + j0 + HALF], w(j0))
                        for j in DVE_TAPS[1:]:
                            y2 = ps.tile([P, HALF], f16, tag="y")
                            nc.vector.scalar_tensor_tensor(
                                y2[:], X[:, base + j:base + j + HALF], w(j),
                                y[:], mult, add)
                            y = y2
                        o = po.tile([P, HALF], f16, tag="o")
                        nc.vector.tensor_tensor(o[:], y[:], s[:], add)
                        nc.sync.dma_start(out[blk, :, bb, base:base + HALF],
                                          o[:])
    nc.compile()
    return nc


def _prepare(x, w, b):
    x = np.asarray(x, dtype=np.float32)
    w = np.asarray(w, dtype=np.float32)
    b = np.asarray(b, dtype=np.float32)
    npe = len(PE_TAPS)
    # channel-major, left zero-padded time, fp16: [D, B, TP]
    xp = np.zeros((D, B, TP), dtype=np.float16)
    xp[:, :, K - 1:] = x.transpose(2, 0, 1)
    wbt = np.concatenate([w.T, b[:, None]], axis=1).astype(np.float32)  # [D, 5]
    in_maps = []
    for m in range(N_CORES):
        sl = slice(m * D_SH, (m + 1) * D_SH)
        wbm = wbt[sl].reshape(N_BLK, P, W_STRIDE).transpose(1, 0, 2)
        # diag stationary matrices for the PE taps: [P, n_blk*npe*P]
        wdm = np.zeros((P, N_BLK, npe, P), dtype=np.float16)
        rng = np.arange(P)
        for blk in range(N_BLK):
            for ti, tap in enumerate(PE_TAPS):
                wdm[rng, blk, ti, rng] = w[tap, m * D_SH + blk * P + rng]
        in_maps.append({
            "x": np.ascontiguousarray(xp[sl]).reshape(N_BLK, P, B, TP),
            "wd": np.ascontiguousarray(wdm).reshape(P, N_BLK * npe * P),
            "wb": np.ascontiguousarray(wbm).reshape(P, N_BLK * W_STRIDE),
        })
    return in_maps


def _collect(results):
    out = np.empty((B, T, D), dtype=np.float32)
    for m in range(N_CORES):
        o = np.asarray(results[m]["out"]).astype(np.float32)
        o = o.reshape(D_SH, B, T)
        out[:, :, m * D_SH:(m + 1) * D_SH] = o.transpose(1, 2, 0)
    return out


def _run(in_maps, trace=False, **kwargs):
    nc = _build()
    return bass_utils.run_bass_kernel_spmd(
        nc, in_maps, core_ids=list(range(N_CORES)), trace=trace, **kwargs)


def kernel(x, w, b):
    in_maps = _prepare(x, w, b)
    try:
        res = _run(in_maps)
    except Exception:
        # Transient NRT device errors have been observed on a cold first
        # execute; one retry (fresh compile dir) clears them.
        res = _run(in_maps)
    return _collect(res.results)
